# revision 5
# baseline (speedup 1.0000x reference)
"""MatchingNetworks forward as a Trainium2 Bass/Tile kernel, SPMD over 8 cores.

Math (per the reference):
    qe = Xq @ W + b            [Q, 64]
    se = Xs @ W + b            [S, 64]
    sims = l2n(qe) @ l2n(se).T [Q, S]
    attn = softmax(sims, axis=1)
    out  = attn @ one_hot(labels, 20)

Sharding: data-parallel over Q (512 queries per core); support set, weights
and bias replicated. Each core is fully independent (no collectives).

Device layout: the contraction dim D_in=21168 must live on SBUF partitions
for the TensorE matmuls, so the host pre-packs W|Xs^T|Xq^T K-tile-interleaved
into ONE combined [128, T*676] bf16 buffer (per K-tile t: 64 cols of W,
100 cols of Xs^T, 512 cols of Xq^T; partition p holds contraction row
t*128+p). One buffer means one DMA per chunk of K-tiles, which gives
fully-contiguous per-partition reads. bf16 halves the HBM traffic (the
kernel is DMA-bound); PSUM accumulation is fp32 and everything after the
embedding GEMM is fp32.

The embedding GEMMs compute emb^T = W.T @ X^T ([64, n] PSUM accumulators
over 166 K-tiles). The bias is added during the PSUM->SBUF activations
(bias is per-partition in the emb^T orientation). Norms use ones-vector
matmuls; softmax runs in [q, s] orientation on ACT/DVE; the label
segment-sum is a one_hot matmul in transposed orientation with small PE
transposes.

This walrus build rejects any instruction carrying MORE THAN ONE sync wait,
so the kernel is structured to keep every instruction at <=1 un-covered
cross-engine dependency:
  - one DMA per K-chunk (all three operands in one buffer);
  - tiny [1,1] "absorber" SBUF->SBUF DMAs on the sync engine advance its
    vector clock past the previous slot-writer before each reused-slot
    chunk DMA (leaving only the WAR-on-PE wait for the real DMA);
  - 1x1 "touch" matmuls pre-absorb the identity / one_hot waits on PE;
  - only ACT reads PSUM (PSUM-slot WAR then merges with ACT data deps);
  - DVE outputs reach ACT consumers directly, but PE/DMA consumers only
    via ACT copies;
  - tail SBUF tiles use bufs=4 (one per q-subtile) so no slot is reused.
"""

import sys

sys.path.insert(0, "/opt/trn_rl_repo")

import numpy as np
import ml_dtypes

import concourse.bass as bass
import concourse.mybir as mybir
import concourse.tile as tile
from concourse.bass_utils import run_bass_kernel_spmd
from concourse.masks import make_identity

# Problem constants (hardcoded per the grading contract).
S = 100
Q = 4096
D_IN = 21168
D = 64
NWAY = 20
NCORES = 8
QC = Q // NCORES  # 512 queries per core
KP = 128
T = (D_IN + KP - 1) // KP  # 166 K-tiles
KPAD = T * KP  # 21248
CH = 16  # K-tiles per DMA chunk
TW = D + S + QC  # 676 columns per K-tile in the combined buffer
NBUF = 3  # chunk double-buffering depth

F32 = mybir.dt.float32
BF16 = mybir.dt.bfloat16
BF16_NP = ml_dtypes.bfloat16


def _chunks(total, ch):
    t0 = 0
    while t0 < total:
        c = min(ch, total - t0)
        yield t0, c
        t0 += c


def build_bass(t_tiles=T):
    """Build the per-core Bass program. t_tiles shrinks the K extent for sim."""
    nc = bass.Bass()
    data = nc.dram_tensor("data", [KP, t_tiles * TW], BF16, kind="ExternalInput")
    bias = nc.dram_tensor("bias", [D, 1], F32, kind="ExternalInput")
    onehot = nc.dram_tensor("onehot", [S, NWAY], F32, kind="ExternalInput")
    out = nc.dram_tensor("out", [QC, NWAY], F32, kind="ExternalOutput")

    AF = mybir.ActivationFunctionType
    AX = mybir.AxisListType

    with tile.TileContext(nc) as tc:
        with (
            tc.tile_pool(name="const", bufs=1) as const,
            tc.tile_pool(name="stream", bufs=NBUF) as stream,
            tc.tile_pool(name="scr", bufs=1) as scr,
            tc.tile_pool(name="sb", bufs=4) as sb,
            tc.tile_pool(name="ps_q", bufs=1, space="PSUM") as ps_q,
            tc.tile_pool(name="ps_s", bufs=1, space="PSUM") as ps_s,
            tc.tile_pool(name="ps_t", bufs=4, space="PSUM") as ps_t,
            tc.tile_pool(name="ps_d", bufs=1, space="PSUM") as ps_d,
        ):
            ident = const.tile([128, 128], F32)
            make_identity(nc, ident[:])
            bias_col = const.tile([D, 1], F32)
            nc.sync.dma_start(bias_col[:], bias[:])
            oh_sb = const.tile([S, NWAY], F32)
            nc.sync.dma_start(oh_sb[:], onehot[:])

            # ones column built on ACT (absorbs the bias DMA wait and, via the
            # Exp's float-bias const conversion, the const-region wait).
            ones_col = const.tile([D, 1], F32)
            nc.scalar.mul(ones_col[:], bias_col[:], 0.0)
            nc.scalar.activation(ones_col[:], ones_col[:], AF.Exp)  # exp(0) = 1

            # 1x1 touch matmuls: absorb ident (gpsimd) and onehot (DMA lane)
            # waits on the PE clock so later Matmults carry <=1 wait.
            dummy_ps = ps_d.tile([1, 1], F32)
            nc.tensor.matmul(
                dummy_ps[:], lhsT=ident[:1, :1], rhs=ident[:1, :1], start=True, stop=True
            )
            nc.tensor.matmul(
                dummy_ps[:], lhsT=oh_sb[:1, :1], rhs=oh_sb[:1, :1], start=True, stop=True
            )

            # PSUM accumulators for emb^T.
            embq_ps = ps_q.tile([D, QC], F32)
            sembT_ps = ps_s.tile([D, S], F32)

            # Main K loop: one DMA per chunk of K-tiles, accumulate both GEMMs.
            chunk_tiles = []
            for ci, (t0, csz) in enumerate(_chunks(t_tiles, CH)):
                if ci >= NBUF:
                    # Absorb the previous slot-writer's completion on the sync
                    # engine's clock so the chunk DMA below needs only its
                    # WAR-on-PE wait.
                    scratch = scr.tile([1, 1], BF16, tag=f"scr{ci}")
                    nc.sync.dma_start(scratch[:], chunk_tiles[ci - NBUF][:1, :1])
                chunk = stream.tile([KP, csz * TW], BF16, tag="chunk")
                chunk_tiles.append(chunk)
                nc.sync.dma_start(chunk[:], data[:, t0 * TW : (t0 + csz) * TW])
                for i in range(csz):
                    t = t0 + i
                    base = i * TW
                    wt = chunk[:, base : base + D]
                    xst = chunk[:, base + D : base + D + S]
                    xqt = chunk[:, base + D + S : base + TW]
                    nc.tensor.matmul(
                        embq_ps[:], lhsT=wt, rhs=xqt, start=(t == 0), stop=(t == t_tiles - 1)
                    )
                    nc.tensor.matmul(
                        sembT_ps[:], lhsT=wt, rhs=xst, start=(t == 0), stop=(t == t_tiles - 1)
                    )

            # ---- support tail: bias add + normalize the support embeddings ----
            sT = sb.tile([D, S], F32, bufs=1)
            nc.scalar.activation(sT[:], sembT_ps[:], AF.Identity, bias=bias_col[:])
            ssq = sb.tile([D, S], F32, bufs=1)
            nc.scalar.activation(ssq[:], sembT_ps[:], AF.Square, bias=bias_col[:])
            ns_ps = ps_t.tile([S, 1], F32, tag="tp")
            nc.tensor.matmul(ns_ps[:], lhsT=ssq[:], rhs=ones_col[:], start=True, stop=True)
            nsc = sb.tile([S, 1], F32, bufs=1)
            nc.scalar.copy(nsc[:], ns_ps[:])
            nsi = sb.tile([S, 1], F32, bufs=1)
            nc.vector.reciprocal(nsi[:], nsc[:])
            cs = sb.tile([S, 1], F32, bufs=1)
            nc.scalar.sqrt(cs[:], nsi[:])  # 1/||se||
            semb_ps = ps_t.tile([S, D], F32, tag="tp")
            nc.tensor.transpose(semb_ps[:], sT[:], ident[:D, :D])
            sn = sb.tile([S, D], F32, bufs=1)
            nc.scalar.mul(sn[:], semb_ps[:], cs[:])
            snT_ps = ps_t.tile([D, S], F32, tag="tp")
            nc.tensor.transpose(snT_ps[:], sn[:], ident[:S, :S])
            snT = sb.tile([D, S], F32, bufs=1)
            nc.scalar.copy(snT[:], snT_ps[:])

            # ---- query tail: norms, cosine sims, softmax, label segment-sum ----
            qT = sb.tile([D, QC], F32, bufs=1)
            nc.scalar.activation(qT[:], embq_ps[:], AF.Identity, bias=bias_col[:])
            sqT = sb.tile([D, QC], F32, bufs=1)
            nc.scalar.activation(sqT[:], embq_ps[:], AF.Square, bias=bias_col[:])

            for j in range(QC // 128):
                qs = slice(j * 128, (j + 1) * 128)
                nq_ps = ps_t.tile([128, 1], F32, tag="tp")
                nc.tensor.matmul(
                    nq_ps[:], lhsT=sqT[:, qs], rhs=ones_col[:], start=True, stop=True
                )
                nqc = sb.tile([128, 1], F32, tag="nqc")
                nc.scalar.copy(nqc[:], nq_ps[:])
                nqi = sb.tile([128, 1], F32, tag="nqi")
                nc.vector.reciprocal(nqi[:], nqc[:])
                aq = sb.tile([128, 1], F32, tag="aq")
                nc.scalar.sqrt(aq[:], nqi[:])  # 1/||qe||
                sims_ps = ps_t.tile([128, S], F32, tag="tp")
                nc.tensor.matmul(
                    sims_ps[:], lhsT=qT[:, qs], rhs=snT[:], start=True, stop=True
                )
                sims = sb.tile([128, S], F32, tag="sims")
                nc.scalar.mul(sims[:], sims_ps[:], aq[:])
                negmax = sb.tile([128, 1], F32, tag="negmax")
                nc.vector.reduce_max(negmax[:], sims[:], axis=AX.X, negate=True)
                etile = sb.tile([128, S], F32, tag="etile")
                denom = sb.tile([128, 1], F32, tag="denom")
                nc.scalar.activation(
                    etile[:], sims[:], AF.Exp, bias=negmax[:], accum_out=denom[:]
                )
                rden = sb.tile([128, 1], F32, tag="rden")
                nc.vector.reciprocal(rden[:], denom[:])
                rdenc = sb.tile([128, 1], F32, tag="rdenc")
                nc.scalar.copy(rdenc[:], rden[:])
                eT_ps = ps_t.tile([S, 128], F32, tag="tp")
                nc.tensor.transpose(eT_ps[:], etile[:], ident[:])
                eT = sb.tile([S, 128], F32, tag="eT")
                nc.scalar.copy(eT[:], eT_ps[:])
                numT_ps = ps_t.tile([NWAY, 128], F32, tag="tp")
                nc.tensor.matmul(
                    numT_ps[:], lhsT=oh_sb[:], rhs=eT[:], start=True, stop=True
                )
                numT = sb.tile([NWAY, 128], F32, tag="numT")
                nc.scalar.copy(numT[:], numT_ps[:])
                num_ps = ps_t.tile([128, NWAY], F32, tag="tp")
                nc.tensor.transpose(num_ps[:], numT[:], ident[:NWAY, :NWAY])
                lg = sb.tile([128, NWAY], F32, tag="lg")
                nc.scalar.mul(lg[:], num_ps[:], rdenc[:])
                nc.sync.dma_start(out[qs, :], lg[:])
    return nc


def legalize_single_wait(nc):
    """Split multi-wait instructions: this walrus build allows at most ONE
    sync wait per instruction, so hoist extra waits onto same-engine NoOps
    inserted immediately before the instruction (identical semantics: the
    engine executes its queue in order)."""
    import bass_rust

    ctr = 0
    nsplit = 0
    for f in nc.m.functions:
        for bb in f.blocks:
            il = bb.instructions
            i = 0
            while i < len(il):
                ins = il[i]
                si = getattr(ins, "sync_info", None)
                if si is not None and len(si.on_wait) > 1:
                    waits = list(si.on_wait)
                    for w in waits[:-1]:
                        nop = bass_rust.InstNoOp(name=f"W-split-{ctr}")
                        ctr += 1
                        nop.engine = ins.engine
                        nop.sync_info = bass_rust.SyncInfo(on_wait=[w], on_update=[])
                        il.insert(i, nop)
                        i += 1
                    ins.sync_info = bass_rust.SyncInfo(
                        on_wait=[waits[-1]], on_update=list(si.on_update)
                    )
                    nsplit += 1
                i += 1
    # verify the rewrite took (bb.instructions must be a live list)
    remaining = sum(
        1
        for f in nc.m.functions
        for bb in f.blocks
        for ins in bb.instructions
        if getattr(ins, "sync_info", None) is not None
        and len(ins.sync_info.on_wait) > 1
    )
    assert remaining == 0, f"legalize_single_wait: {remaining} multi-wait instrs left"
    return nc


def pack_combined(W_, Xs_, Xq_core, t_tiles=T):
    """Build the combined [128, t_tiles*TW] bf16 buffer.

    Per K-tile t (contraction rows t*128..t*128+127):
      cols [0:64)    = W rows           (w_t[p, j]  = W[t*128+p, j])
      cols [64:164)  = Xs^T rows        (xs_t[p, j] = Xs[j, t*128+p])
      cols [164:676) = Xq_core^T rows   (xq_t[p, j] = Xq[j, t*128+p])
    """
    kext = t_tiles * KP
    A = np.zeros((KP, t_tiles, TW), dtype=BF16_NP)

    Wp = np.zeros((kext, D), dtype=BF16_NP)
    Wp[: W_.shape[0]] = W_.astype(BF16_NP)
    A[:, :, :D] = Wp.reshape(t_tiles, KP, D).transpose(1, 0, 2)

    def rows_pack(X, n):
        Xp = np.zeros((n, kext), dtype=BF16_NP)
        Xp[:, : X.shape[1]] = X.astype(BF16_NP)
        return Xp.reshape(n, t_tiles, KP).transpose(2, 1, 0)  # [128, t, n]

    A[:, :, D : D + S] = rows_pack(Xs_, S)
    A[:, :, D + S :] = rows_pack(Xq_core, QC)
    return np.ascontiguousarray(A.reshape(KP, t_tiles * TW))


def make_in_maps(support_images, support_labels, query_images, backbone_w, backbone_b):
    Xq = np.asarray(query_images, dtype=np.float32)
    Xs = np.asarray(support_images, dtype=np.float32)
    W = np.asarray(backbone_w, dtype=np.float32)
    b = np.asarray(backbone_b, dtype=np.float32).reshape(D, 1)
    labels = np.asarray(support_labels).astype(np.int64).reshape(S)
    onehot = np.zeros((S, NWAY), np.float32)
    onehot[np.arange(S), labels] = 1.0

    common = {"bias": b, "onehot": onehot}
    in_maps = []
    for c in range(NCORES):
        data_c = pack_combined(W, Xs, Xq[c * QC : (c + 1) * QC])
        in_maps.append({"data": data_c, **common})
    return in_maps


def run(in_maps, trace=False, **kw):
    nc = build_bass()
    legalize_single_wait(nc)
    return run_bass_kernel_spmd(nc, in_maps, list(range(NCORES)), trace=trace, **kw)


def kernel(
    support_images,
    support_labels,
    query_images,
    n_way,
    k_shot,
    backbone_w,
    backbone_b,
):
    assert int(n_way) == NWAY
    in_maps = make_in_maps(
        support_images, support_labels, query_images, backbone_w, backbone_b
    )
    res = run(in_maps, trace=False)
    return np.concatenate(
        [np.asarray(res.results[c]["out"]) for c in range(NCORES)], axis=0
    )


# revision 12
# speedup vs baseline: 1.0480x; 1.0480x over previous
"""MatchingNetworks forward as a Trainium2 Bass/Tile kernel, SPMD over 8 cores.

Math (per the reference):
    qe = Xq @ W + b            [Q, 64]
    se = Xs @ W + b            [S, 64]
    sims = l2n(qe) @ l2n(se).T [Q, S]
    attn = softmax(sims, axis=1)
    out  = attn @ one_hot(labels, 20)

Sharding: data-parallel over Q (512 queries per core); support set, weights
and bias replicated. Each core is fully independent (no collectives).

Device layout: the contraction dim D_in=21168 must live on SBUF partitions
for the TensorE matmuls, so the host pre-packs W|Xs^T|Xq^T K-tile-interleaved
into ONE combined [128, T*676] bf16 buffer (per K-tile t: 64 cols of W,
100 cols of Xs^T, 512 cols of Xq^T; partition p holds contraction row
t*128+p). One buffer means one DMA per chunk of K-tiles, which gives
fully-contiguous per-partition reads. bf16 halves the HBM traffic (the
kernel is DMA-bound); PSUM accumulation is fp32 and everything after the
embedding GEMM is fp32.

The embedding GEMMs compute emb^T = W.T @ X^T ([64, n] PSUM accumulators
over 166 K-tiles). The bias is added during the PSUM->SBUF activations
(bias is per-partition in the emb^T orientation). Norms use ones-vector
matmuls; softmax runs in [q, s] orientation on ACT/DVE; the label
segment-sum is a one_hot matmul in transposed orientation with small PE
transposes.

This walrus build rejects any instruction carrying MORE THAN ONE sync wait,
so the kernel is structured to keep every instruction at <=1 un-covered
cross-engine dependency:
  - one DMA per K-chunk (all three operands in one buffer);
  - tiny [1,1] "absorber" SBUF->SBUF DMAs on the sync engine advance its
    vector clock past the previous slot-writer before each reused-slot
    chunk DMA (leaving only the WAR-on-PE wait for the real DMA);
  - 1x1 "touch" matmuls pre-absorb the identity / one_hot waits on PE;
  - only ACT reads PSUM (PSUM-slot WAR then merges with ACT data deps);
  - DVE outputs reach ACT consumers directly, but PE/DMA consumers only
    via ACT copies;
  - tail SBUF tiles use bufs=4 (one per q-subtile) so no slot is reused.
"""

import sys

sys.path.insert(0, "/opt/trn_rl_repo")

import numpy as np
import ml_dtypes

import concourse.bass as bass
import concourse.mybir as mybir
import concourse.tile as tile
from concourse.bass_utils import run_bass_kernel_spmd
from concourse.masks import make_identity

# Problem constants (hardcoded per the grading contract).
S = 100
Q = 4096
D_IN = 21168
D = 64
NWAY = 20
NCORES = 8
QC = Q // NCORES  # 512 queries per core
KP = 128
T = (D_IN + KP - 1) // KP  # 166 K-tiles
KPAD = T * KP  # 21248
CH = 8  # K-tiles per DMA chunk
TW = D + S + QC  # 676 columns per K-tile in the combined buffer
NBUF = 4  # chunk double-buffering depth

F32 = mybir.dt.float32
BF16 = mybir.dt.bfloat16
BF16_NP = ml_dtypes.bfloat16


def _chunks(total, ch):
    t0 = 0
    while t0 < total:
        c = min(ch, total - t0)
        yield t0, c
        t0 += c


def build_bass(t_tiles=T):
    """Build the per-core Bass program. t_tiles shrinks the K extent for sim."""
    nc = bass.Bass()
    data = nc.dram_tensor("data", [KP, t_tiles * TW], BF16, kind="ExternalInput")
    bias = nc.dram_tensor("bias", [D, 1], F32, kind="ExternalInput")
    onehot = nc.dram_tensor("onehot", [S, NWAY], F32, kind="ExternalInput")
    out = nc.dram_tensor("out", [QC, NWAY], F32, kind="ExternalOutput")

    AF = mybir.ActivationFunctionType
    AX = mybir.AxisListType

    with tile.TileContext(nc) as tc:
        with (
            tc.tile_pool(name="const", bufs=1) as const,
            tc.tile_pool(name="stream", bufs=NBUF) as stream,
            tc.tile_pool(name="scr", bufs=1) as scr,
            tc.tile_pool(name="sb", bufs=4) as sb,
            tc.tile_pool(name="ps_q", bufs=1, space="PSUM") as ps_q,
            tc.tile_pool(name="ps_t", bufs=4, space="PSUM") as ps_t,
            tc.tile_pool(name="ps_d", bufs=1, space="PSUM") as ps_d,
        ):
            ident = const.tile([128, 128], F32)
            make_identity(nc, ident[:])
            bias_col = const.tile([D, 1], F32)
            nc.sync.dma_start(bias_col[:], bias[:])
            oh_sb = const.tile([S, NWAY], F32)
            nc.sync.dma_start(oh_sb[:], onehot[:])

            # ones column built on ACT (absorbs the bias DMA wait and, via the
            # Exp's float-bias const conversion, the const-region wait).
            ones_col = const.tile([D, 1], F32)
            nc.scalar.mul(ones_col[:], bias_col[:], 0.0)
            nc.scalar.activation(ones_col[:], ones_col[:], AF.Exp)  # exp(0) = 1

            # 1x1 touch matmuls: absorb ident (gpsimd) and onehot (DMA lane)
            # waits on the PE clock so later Matmults carry <=1 wait.
            dummy_ps = ps_d.tile([1, 1], F32)
            nc.tensor.matmul(
                dummy_ps[:], lhsT=ident[:1, :1], rhs=ident[:1, :1], start=True, stop=True
            )
            nc.tensor.matmul(
                dummy_ps[:], lhsT=oh_sb[:1, :1], rhs=oh_sb[:1, :1], start=True, stop=True
            )

            # One shared PSUM bank for both emb^T accumulators: the query GEMM
            # runs on array columns 0-63 (out partitions 0-63), the support
            # GEMM on columns 64-127 (out partitions 64-127) via tile_position.
            # The two matmuls per K-tile then overlap inside the PE array and
            # their weight loads hide in the 64-deep reorder window.
            acc = ps_q.tile([KP, QC], F32)
            embq_ps = acc[:D, :]
            sembT_ps = acc[D : 2 * D, :S]

            # Main K loop: one DMA per chunk of K-tiles, accumulate both GEMMs.
            chunk_tiles = []
            for ci, (t0, csz) in enumerate(_chunks(t_tiles, CH)):
                if ci >= NBUF:
                    # Absorb the previous slot-writer's completion on the sync
                    # engine's clock so the chunk DMA below needs only its
                    # WAR-on-PE wait.
                    scratch = scr.tile([1, 1], BF16, tag=f"scr{ci}")
                    nc.sync.dma_start(scratch[:], chunk_tiles[ci - NBUF][:1, :1])
                chunk = stream.tile([KP, csz * TW], BF16, tag="chunk")
                chunk_tiles.append(chunk)
                nc.sync.dma_start(chunk[:], data[:, t0 * TW : (t0 + csz) * TW])
                for i in range(csz):
                    t = t0 + i
                    base = i * TW
                    wt = chunk[:, base : base + D]
                    xst = chunk[:, base + D : base + D + S]
                    xqt = chunk[:, base + D + S : base + TW]
                    nc.tensor.matmul(
                        embq_ps,
                        lhsT=wt,
                        rhs=xqt,
                        start=(t == 0),
                        stop=(t == t_tiles - 1),
                        tile_position=(0, 0),
                        skip_group_check=True,
                    )
                    nc.tensor.matmul(
                        sembT_ps,
                        lhsT=wt,
                        rhs=xst,
                        start=(t == 0),
                        stop=(t == t_tiles - 1),
                        tile_position=(0, D),
                        skip_group_check=True,
                    )

            # ---- support tail: bias add + normalize the support embeddings ----
            sT = sb.tile([D, S], F32, bufs=1)
            nc.scalar.activation(sT[:], sembT_ps, AF.Identity, bias=bias_col[:])
            ssq = sb.tile([D, S], F32, bufs=1)
            nc.scalar.activation(ssq[:], sembT_ps, AF.Square, bias=bias_col[:])
            ns_ps = ps_t.tile([S, 1], F32, tag="tp")
            nc.tensor.matmul(ns_ps[:], lhsT=ssq[:], rhs=ones_col[:], start=True, stop=True)
            nsc = sb.tile([S, 1], F32, bufs=1)
            nc.vector.tensor_copy(nsc[:], ns_ps[:])
            nsi = sb.tile([S, 1], F32, bufs=1)
            nc.vector.reciprocal(nsi[:], nsc[:])
            cs = sb.tile([S, 1], F32, bufs=1)
            nc.scalar.sqrt(cs[:], nsi[:])  # 1/||se||
            semb_ps = ps_t.tile([S, D], F32, tag="tp")
            nc.tensor.transpose(semb_ps[:], sT[:], ident[:D, :D])
            sn = sb.tile([S, D], F32, bufs=1)
            nc.scalar.mul(sn[:], semb_ps[:], cs[:])
            snT_ps = ps_t.tile([D, S], F32, tag="tp")
            nc.tensor.transpose(snT_ps[:], sn[:], ident[:S, :S])
            snT = sb.tile([D, S], F32, bufs=1)
            nc.vector.tensor_copy(snT[:], snT_ps[:])

            # ---- query tail: norms, cosine sims, softmax, label segment-sum ----
            qT = sb.tile([D, QC], F32, bufs=1)
            nc.scalar.activation(qT[:], embq_ps, AF.Identity, bias=bias_col[:])
            sqT = sb.tile([D, QC], F32, bufs=1)
            nc.scalar.activation(sqT[:], embq_ps, AF.Square, bias=bias_col[:])

            for j in range(QC // 128):
                qs = slice(j * 128, (j + 1) * 128)
                nq_ps = ps_t.tile([128, 1], F32, tag="tp")
                nc.tensor.matmul(
                    nq_ps[:], lhsT=sqT[:, qs], rhs=ones_col[:], start=True, stop=True
                )
                nqi = sb.tile([128, 1], F32, tag="nqi")
                nc.vector.reciprocal(nqi[:], nq_ps[:])
                aq = sb.tile([128, 1], F32, tag="aq")
                nc.scalar.sqrt(aq[:], nqi[:])  # 1/||qe||
                sims_ps = ps_t.tile([128, S], F32, tag="tp")
                nc.tensor.matmul(
                    sims_ps[:], lhsT=qT[:, qs], rhs=snT[:], start=True, stop=True
                )
                sims = sb.tile([128, S], F32, tag="sims")
                nc.scalar.mul(sims[:], sims_ps[:], aq[:])
                negmax = sb.tile([128, 1], F32, tag="negmax")
                nc.vector.reduce_max(negmax[:], sims[:], axis=AX.X, negate=True)
                etile = sb.tile([128, S], F32, tag="etile")
                denom = sb.tile([128, 1], F32, tag="denom")
                nc.scalar.activation(
                    etile[:], sims[:], AF.Exp, bias=negmax[:], accum_out=denom[:]
                )
                rden = sb.tile([128, 1], F32, tag="rden")
                nc.vector.reciprocal(rden[:], denom[:])
                eT_ps = ps_t.tile([S, 128], F32, tag="tp")
                nc.tensor.transpose(eT_ps[:], etile[:], ident[:])
                eT = sb.tile([S, 128], F32, tag="eT")
                nc.vector.tensor_copy(eT[:], eT_ps[:])
                numT_ps = ps_t.tile([NWAY, 128], F32, tag="tp")
                nc.tensor.matmul(
                    numT_ps[:], lhsT=oh_sb[:], rhs=eT[:], start=True, stop=True
                )
                numT = sb.tile([NWAY, 128], F32, tag="numT")
                nc.vector.tensor_copy(numT[:], numT_ps[:])
                num_ps = ps_t.tile([128, NWAY], F32, tag="tp")
                nc.tensor.transpose(num_ps[:], numT[:], ident[:NWAY, :NWAY])
                lg = sb.tile([128, NWAY], F32, tag="lg")
                nc.vector.tensor_scalar_mul(lg[:], num_ps[:], rden[:])
                nc.sync.dma_start(out[qs, :], lg[:])
    return nc


def legalize_single_wait(nc):
    """Split multi-wait instructions: this walrus build allows at most ONE
    sync wait per instruction, so hoist extra waits onto same-engine NoOps
    inserted immediately before the instruction (identical semantics: the
    engine executes its queue in order)."""
    import bass_rust

    ctr = 0
    nsplit = 0
    for f in nc.m.functions:
        for bb in f.blocks:
            il = bb.instructions
            i = 0
            while i < len(il):
                ins = il[i]
                si = getattr(ins, "sync_info", None)
                if si is not None and len(si.on_wait) > 1:
                    waits = list(si.on_wait)
                    for w in waits[:-1]:
                        nop = bass_rust.InstNoOp(name=f"W-split-{ctr}")
                        ctr += 1
                        nop.engine = ins.engine
                        nop.sync_info = bass_rust.SyncInfo(on_wait=[w], on_update=[])
                        il.insert(i, nop)
                        i += 1
                    ins.sync_info = bass_rust.SyncInfo(
                        on_wait=[waits[-1]], on_update=list(si.on_update)
                    )
                    nsplit += 1
                i += 1
    # verify the rewrite took (bb.instructions must be a live list)
    remaining = sum(
        1
        for f in nc.m.functions
        for bb in f.blocks
        for ins in bb.instructions
        if getattr(ins, "sync_info", None) is not None
        and len(ins.sync_info.on_wait) > 1
    )
    assert remaining == 0, f"legalize_single_wait: {remaining} multi-wait instrs left"
    return nc


def pack_combined(W_, Xs_, Xq_core, t_tiles=T):
    """Build the combined [128, t_tiles*TW] bf16 buffer.

    Per K-tile t (contraction rows t*128..t*128+127):
      cols [0:64)    = W rows           (w_t[p, j]  = W[t*128+p, j])
      cols [64:164)  = Xs^T rows        (xs_t[p, j] = Xs[j, t*128+p])
      cols [164:676) = Xq_core^T rows   (xq_t[p, j] = Xq[j, t*128+p])
    """
    kext = t_tiles * KP
    A = np.zeros((KP, t_tiles, TW), dtype=BF16_NP)

    Wp = np.zeros((kext, D), dtype=BF16_NP)
    Wp[: W_.shape[0]] = W_.astype(BF16_NP)
    A[:, :, :D] = Wp.reshape(t_tiles, KP, D).transpose(1, 0, 2)

    def rows_pack(X, n):
        Xp = np.zeros((n, kext), dtype=BF16_NP)
        Xp[:, : X.shape[1]] = X.astype(BF16_NP)
        return Xp.reshape(n, t_tiles, KP).transpose(2, 1, 0)  # [128, t, n]

    A[:, :, D : D + S] = rows_pack(Xs_, S)
    A[:, :, D + S :] = rows_pack(Xq_core, QC)
    return np.ascontiguousarray(A.reshape(KP, t_tiles * TW))


def make_in_maps(support_images, support_labels, query_images, backbone_w, backbone_b):
    Xq = np.asarray(query_images, dtype=np.float32)
    Xs = np.asarray(support_images, dtype=np.float32)
    W = np.asarray(backbone_w, dtype=np.float32)
    b = np.asarray(backbone_b, dtype=np.float32).reshape(D, 1)
    labels = np.asarray(support_labels).astype(np.int64).reshape(S)
    onehot = np.zeros((S, NWAY), np.float32)
    onehot[np.arange(S), labels] = 1.0

    common = {"bias": b, "onehot": onehot}
    in_maps = []
    for c in range(NCORES):
        data_c = pack_combined(W, Xs, Xq[c * QC : (c + 1) * QC])
        in_maps.append({"data": data_c, **common})
    return in_maps


def run(in_maps, trace=False, **kw):
    nc = build_bass()
    legalize_single_wait(nc)
    return run_bass_kernel_spmd(nc, in_maps, list(range(NCORES)), trace=trace, **kw)


def kernel(
    support_images,
    support_labels,
    query_images,
    n_way,
    k_shot,
    backbone_w,
    backbone_b,
):
    assert int(n_way) == NWAY
    in_maps = make_in_maps(
        support_images, support_labels, query_images, backbone_w, backbone_b
    )
    res = run(in_maps, trace=False)
    return np.concatenate(
        [np.asarray(res.results[c]["out"]) for c in range(NCORES)], axis=0
    )


# revision 14
# speedup vs baseline: 1.1856x; 1.1313x over previous
"""MatchingNetworks forward as a Trainium2 Bass/Tile kernel, SPMD over 8 cores.

Math (per the reference):
    qe = Xq @ W + b            [Q, 64]
    se = Xs @ W + b            [S, 64]
    sims = l2n(qe) @ l2n(se).T [Q, S]
    attn = softmax(sims, axis=1)
    out  = attn @ one_hot(labels, 20)

Sharding: data-parallel over Q (512 queries per core); support set, weights
and bias replicated. Each core is fully independent (no collectives).

Device layout: the contraction dim D_in=21168 must live on SBUF partitions
for the TensorE matmuls, so the host pre-packs W|Xs^T|Xq^T K-tile-interleaved
into ONE combined [128, T*676] bf16 buffer (per K-tile t: 64 cols of W,
100 cols of Xs^T, 512 cols of Xq^T; partition p holds contraction row
t*128+p). One buffer means one DMA per chunk of K-tiles, which gives
fully-contiguous per-partition reads. bf16 halves the HBM traffic (the
kernel is DMA-bound); PSUM accumulation is fp32 and everything after the
embedding GEMM is fp32.

The embedding GEMMs compute emb^T = W.T @ X^T ([64, n] PSUM accumulators
over 166 K-tiles). The bias is added during the PSUM->SBUF activations
(bias is per-partition in the emb^T orientation). Norms use ones-vector
matmuls; softmax runs in [q, s] orientation on ACT/DVE; the label
segment-sum is a one_hot matmul in transposed orientation with small PE
transposes.

This walrus build rejects any instruction carrying MORE THAN ONE sync wait,
so the kernel is structured to keep every instruction at <=1 un-covered
cross-engine dependency:
  - one DMA per K-chunk (all three operands in one buffer);
  - tiny [1,1] "absorber" SBUF->SBUF DMAs on the sync engine advance its
    vector clock past the previous slot-writer before each reused-slot
    chunk DMA (leaving only the WAR-on-PE wait for the real DMA);
  - 1x1 "touch" matmuls pre-absorb the identity / one_hot waits on PE;
  - only ACT reads PSUM (PSUM-slot WAR then merges with ACT data deps);
  - DVE outputs reach ACT consumers directly, but PE/DMA consumers only
    via ACT copies;
  - tail SBUF tiles use bufs=4 (one per q-subtile) so no slot is reused.
"""

import sys

sys.path.insert(0, "/opt/trn_rl_repo")

import numpy as np
import ml_dtypes

import concourse.bass as bass
import concourse.mybir as mybir
import concourse.tile as tile
from concourse.bass_utils import run_bass_kernel_spmd

# Problem constants (hardcoded per the grading contract).
S = 100
Q = 4096
D_IN = 21168
D = 64
NWAY = 20
NCORES = 8
QC = Q // NCORES  # 512 queries per core
KP = 128
T = (D_IN + KP - 1) // KP  # 166 K-tiles
KPAD = T * KP  # 21248
CH = 8  # K-tiles per DMA chunk
TW = D + S + QC  # 676 columns per K-tile in the combined buffer
NBUF = 6  # chunk double-buffering depth

F32 = mybir.dt.float32
BF16 = mybir.dt.bfloat16
BF16_NP = ml_dtypes.bfloat16


def _chunks(total, ch):
    t0 = 0
    while t0 < total:
        c = min(ch, total - t0)
        yield t0, c
        t0 += c


def build_bass(t_tiles=T):
    """Build the per-core Bass program. t_tiles shrinks the K extent for sim."""
    nc = bass.Bass()
    data = nc.dram_tensor("data", [KP, t_tiles * TW], BF16, kind="ExternalInput")
    bias = nc.dram_tensor("bias", [D, 1], F32, kind="ExternalInput")
    onehot = nc.dram_tensor("onehot", [S, NWAY], F32, kind="ExternalInput")
    identity = nc.dram_tensor("identity", [KP, KP], F32, kind="ExternalInput")
    ones = nc.dram_tensor("ones", [D, 1], F32, kind="ExternalInput")
    out = nc.dram_tensor("out", [QC, NWAY], F32, kind="ExternalOutput")

    AF = mybir.ActivationFunctionType
    AX = mybir.AxisListType

    with tile.TileContext(nc) as tc:
        with (
            tc.tile_pool(name="const", bufs=1) as const,
            tc.tile_pool(name="stream", bufs=NBUF) as stream,
            tc.tile_pool(name="sb", bufs=4) as sb,
            tc.tile_pool(name="ps_q", bufs=1, space="PSUM") as ps_q,
            tc.tile_pool(name="ps_t", bufs=4, space="PSUM") as ps_t,
        ):
            ident = const.tile([128, 128], F32)
            nc.sync.dma_start(ident[:], identity[:])
            bias_col = const.tile([D, 1], F32)
            nc.sync.dma_start(bias_col[:], bias[:])
            oh_sb = const.tile([S, NWAY], F32)
            nc.sync.dma_start(oh_sb[:], onehot[:])
            ones_col = const.tile([D, 1], F32)
            nc.sync.dma_start(ones_col[:], ones[:])

            # One shared PSUM bank for both emb^T accumulators: the query GEMM
            # runs on array columns 0-63 (out partitions 0-63), the support
            # GEMM on columns 64-127 (out partitions 64-127) via tile_position.
            # The two matmuls per K-tile then overlap inside the PE array and
            # their weight loads hide in the 64-deep reorder window.
            acc = ps_q.tile([KP, QC], F32)
            embq_ps = acc[:D, :]
            sembT_ps = acc[D : 2 * D, :S]

            # Main K loop: one DMA per chunk of K-tiles, accumulate both GEMMs.
            for ci, (t0, csz) in enumerate(_chunks(t_tiles, CH)):
                chunk = stream.tile([KP, csz * TW], BF16, tag="chunk")
                nc.sync.dma_start(chunk[:], data[:, t0 * TW : (t0 + csz) * TW])
                for i in range(csz):
                    t = t0 + i
                    base = i * TW
                    wt = chunk[:, base : base + D]
                    xst = chunk[:, base + D : base + D + S]
                    xqt = chunk[:, base + D + S : base + TW]
                    nc.tensor.matmul(
                        embq_ps,
                        lhsT=wt,
                        rhs=xqt,
                        start=(t == 0),
                        stop=(t == t_tiles - 1),
                        tile_position=(0, 0),
                        skip_group_check=True,
                    )
                    nc.tensor.matmul(
                        sembT_ps,
                        lhsT=wt,
                        rhs=xst,
                        start=(t == 0),
                        stop=(t == t_tiles - 1),
                        tile_position=(0, D),
                        skip_group_check=True,
                    )

            # ---- support tail: bias add + normalize the support embeddings ----
            sT = sb.tile([D, S], F32, bufs=1)
            nc.scalar.activation(sT[:], sembT_ps, AF.Identity, bias=bias_col[:])
            ssq = sb.tile([D, S], F32, bufs=1)
            nc.scalar.activation(ssq[:], sembT_ps, AF.Square, bias=bias_col[:])
            ns_ps = ps_t.tile([S, 1], F32, tag="tp")
            nc.tensor.matmul(ns_ps[:], lhsT=ssq[:], rhs=ones_col[:], start=True, stop=True)
            nsc = sb.tile([S, 1], F32, bufs=1)
            nc.vector.tensor_copy(nsc[:], ns_ps[:])
            nsi = sb.tile([S, 1], F32, bufs=1)
            nc.vector.reciprocal(nsi[:], nsc[:])
            cs = sb.tile([S, 1], F32, bufs=1)
            nc.scalar.sqrt(cs[:], nsi[:])  # 1/||se||
            semb_ps = ps_t.tile([S, D], F32, tag="tp")
            nc.tensor.transpose(semb_ps[:], sT[:], ident[:D, :D])
            sn = sb.tile([S, D], F32, bufs=1)
            nc.scalar.mul(sn[:], semb_ps[:], cs[:])
            snT_ps = ps_t.tile([D, S], F32, tag="tp")
            nc.tensor.transpose(snT_ps[:], sn[:], ident[:S, :S])
            snT = sb.tile([D, S], F32, bufs=1)
            nc.vector.tensor_copy(snT[:], snT_ps[:])

            # ---- query tail: norms, cosine sims, softmax, label segment-sum ----
            qT = sb.tile([D, QC], F32, bufs=1)
            nc.scalar.activation(qT[:], embq_ps, AF.Identity, bias=bias_col[:])
            sqT = sb.tile([D, QC], F32, bufs=1)
            nc.scalar.activation(sqT[:], embq_ps, AF.Square, bias=bias_col[:])

            for j in range(QC // 128):
                qs = slice(j * 128, (j + 1) * 128)
                nq_ps = ps_t.tile([128, 1], F32, tag="tp")
                nc.tensor.matmul(
                    nq_ps[:], lhsT=sqT[:, qs], rhs=ones_col[:], start=True, stop=True
                )
                nqi = sb.tile([128, 1], F32, tag="nqi")
                nc.vector.reciprocal(nqi[:], nq_ps[:])
                aq = sb.tile([128, 1], F32, tag="aq")
                nc.scalar.sqrt(aq[:], nqi[:])  # 1/||qe||
                sims_ps = ps_t.tile([128, S], F32, tag="tp")
                nc.tensor.matmul(
                    sims_ps[:], lhsT=qT[:, qs], rhs=snT[:], start=True, stop=True
                )
                # exp(sims * 1/||qe||) directly: cosines are in [-1, 1] so the
                # unstabilized softmax cannot overflow.
                etile = sb.tile([128, S], F32, tag="etile")
                denom = sb.tile([128, 1], F32, tag="denom")
                nc.scalar.activation(
                    etile[:], sims_ps[:], AF.Exp, scale=aq[:], accum_out=denom[:]
                )
                rden = sb.tile([128, 1], F32, tag="rden")
                nc.vector.reciprocal(rden[:], denom[:])
                eT_ps = ps_t.tile([S, 128], F32, tag="tp")
                nc.tensor.transpose(eT_ps[:], etile[:], ident[:])
                eT = sb.tile([S, 128], F32, tag="eT")
                nc.vector.tensor_copy(eT[:], eT_ps[:])
                numT_ps = ps_t.tile([NWAY, 128], F32, tag="tp")
                nc.tensor.matmul(
                    numT_ps[:], lhsT=oh_sb[:], rhs=eT[:], start=True, stop=True
                )
                numT = sb.tile([NWAY, 128], F32, tag="numT")
                nc.vector.tensor_copy(numT[:], numT_ps[:])
                num_ps = ps_t.tile([128, NWAY], F32, tag="tp")
                nc.tensor.transpose(num_ps[:], numT[:], ident[:NWAY, :NWAY])
                lg = sb.tile([128, NWAY], F32, tag="lg")
                nc.vector.tensor_scalar_mul(lg[:], num_ps[:], rden[:])
                nc.sync.dma_start(out[qs, :], lg[:])
    return nc


def legalize_single_wait(nc):
    """Split multi-wait instructions: this walrus build allows at most ONE
    sync wait per instruction, so hoist extra waits onto same-engine NoOps
    inserted immediately before the instruction (identical semantics: the
    engine executes its queue in order)."""
    import bass_rust

    ctr = 0
    nsplit = 0
    for f in nc.m.functions:
        for bb in f.blocks:
            il = bb.instructions
            i = 0
            while i < len(il):
                ins = il[i]
                si = getattr(ins, "sync_info", None)
                if si is not None and len(si.on_wait) > 1:
                    waits = list(si.on_wait)
                    for w in waits[:-1]:
                        nop = bass_rust.InstNoOp(name=f"W-split-{ctr}")
                        ctr += 1
                        nop.engine = ins.engine
                        nop.sync_info = bass_rust.SyncInfo(on_wait=[w], on_update=[])
                        il.insert(i, nop)
                        i += 1
                    ins.sync_info = bass_rust.SyncInfo(
                        on_wait=[waits[-1]], on_update=list(si.on_update)
                    )
                    nsplit += 1
                i += 1
    # verify the rewrite took (bb.instructions must be a live list)
    remaining = sum(
        1
        for f in nc.m.functions
        for bb in f.blocks
        for ins in bb.instructions
        if getattr(ins, "sync_info", None) is not None
        and len(ins.sync_info.on_wait) > 1
    )
    assert remaining == 0, f"legalize_single_wait: {remaining} multi-wait instrs left"
    return nc


def pack_combined(W_, Xs_, Xq_core, t_tiles=T):
    """Build the combined [128, t_tiles*TW] bf16 buffer.

    Per K-tile t (contraction rows t*128..t*128+127):
      cols [0:64)    = W rows           (w_t[p, j]  = W[t*128+p, j])
      cols [64:164)  = Xs^T rows        (xs_t[p, j] = Xs[j, t*128+p])
      cols [164:676) = Xq_core^T rows   (xq_t[p, j] = Xq[j, t*128+p])
    """
    kext = t_tiles * KP
    A = np.zeros((KP, t_tiles, TW), dtype=BF16_NP)

    Wp = np.zeros((kext, D), dtype=BF16_NP)
    Wp[: W_.shape[0]] = W_.astype(BF16_NP)
    A[:, :, :D] = Wp.reshape(t_tiles, KP, D).transpose(1, 0, 2)

    def rows_pack(X, n):
        Xp = np.zeros((n, kext), dtype=BF16_NP)
        Xp[:, : X.shape[1]] = X.astype(BF16_NP)
        return Xp.reshape(n, t_tiles, KP).transpose(2, 1, 0)  # [128, t, n]

    A[:, :, D : D + S] = rows_pack(Xs_, S)
    A[:, :, D + S :] = rows_pack(Xq_core, QC)
    return np.ascontiguousarray(A.reshape(KP, t_tiles * TW))


def make_in_maps(support_images, support_labels, query_images, backbone_w, backbone_b):
    Xq = np.asarray(query_images, dtype=np.float32)
    Xs = np.asarray(support_images, dtype=np.float32)
    W = np.asarray(backbone_w, dtype=np.float32)
    b = np.asarray(backbone_b, dtype=np.float32).reshape(D, 1)
    labels = np.asarray(support_labels).astype(np.int64).reshape(S)
    onehot = np.zeros((S, NWAY), np.float32)
    onehot[np.arange(S), labels] = 1.0

    common = {
        "bias": b,
        "onehot": onehot,
        "identity": np.eye(KP, dtype=np.float32),
        "ones": np.ones((D, 1), np.float32),
    }
    in_maps = []
    for c in range(NCORES):
        data_c = pack_combined(W, Xs, Xq[c * QC : (c + 1) * QC])
        in_maps.append({"data": data_c, **common})
    return in_maps


def run(in_maps, trace=False, **kw):
    nc = build_bass()
    legalize_single_wait(nc)
    return run_bass_kernel_spmd(nc, in_maps, list(range(NCORES)), trace=trace, **kw)


def kernel(
    support_images,
    support_labels,
    query_images,
    n_way,
    k_shot,
    backbone_w,
    backbone_b,
):
    assert int(n_way) == NWAY
    in_maps = make_in_maps(
        support_images, support_labels, query_images, backbone_w, backbone_b
    )
    res = run(in_maps, trace=False)
    return np.concatenate(
        [np.asarray(res.results[c]["out"]) for c in range(NCORES)], axis=0
    )
